# revision 15
# baseline (speedup 1.0000x reference)
"""Trainium2 Bass kernel for per-token multi-head self-attention (v5).

Computation (per token t):
  q,k,v = x @ W{q,k,v}.T ; scores = (q_t k_t^T)/sqrt(128) over heads [16x16]
  out_t = softmax(scores) @ v_t ; y = out @ Wo.T

Sharding: data-parallel over the 16384 tokens -> 8 cores x 2048 tokens.

Design:
  * bf16 everywhere on-chip; fp32 PSUM accumulation.
  * Pass A streams Wq/Wk/Wv once (weight-stationary), writes qT/kT
    feature-major and v token-major (v_nat, via PE transposes) to DRAM
    scratch. Loads issue on the sync DMA queue, stores on the scalar
    queue (scalar engine is otherwise idle in pass A).
  * Pass B: 8-token score groups — ONE [128,128] matmul gives a
    group's 16x16 per-token score blocks on the block diagonal;
    cross-token garbage is zeroed by a precomputed mask; the masked
    exp'd tile IS the AV stationary (contraction over (token,g)
    pairs), with a ones column in the moving operand producing the
    softmax normalizer Z. Middle-stage ACT/mask/copy ops are batched
    over QUADS of groups (32 tokens) to amortize their ~0.5us fixed
    costs; the scalar engine carries only exp + the aoT drain copy and
    has no DMA issues in pass B (v4 post-mortem: DMA issue on the
    scalar queue serialized with the critical exp chain).
  * 1/Z normalization is folded into the transpose-back matmul: its
    moving operand is diag(1/Z) (one SBUF-only build op) instead of
    the identity, so out[d,(t,h)] = ao[(t,h),d]/Z[(t,h)] comes out of
    the PE already scaled — no per-group PSUM-read tensor_scalar ops.
  * Wo matmuls for chunk c-1 are interleaved into chunk c's quad loop
    (16 per quad) to keep the PE continuously busy (pstate: 2.4 GHz
    needs ~3us of uninterrupted PE activity).
"""
import math
from collections import deque
from contextlib import ExitStack

import numpy as np

NCORES = 8
E = 2048          # hidden
NH = 16           # heads
HD = 128          # head dim
TPC = 2048        # tokens per core
TC = 512          # token chunk in pass B
P = 128
GS = 8            # tokens per score group
NG = TC // GS     # groups per chunk (64)
UQ = 4            # groups per quad (ACT/mask/copy batch unit)
NQ = NG // UQ     # quads per chunk (16)

_cached = {}


def _build_program():
    import concourse.bass as bass
    import concourse.tile as tile
    from concourse import bacc, mybir

    f32 = mybir.dt.float32
    bf16 = mybir.dt.bfloat16
    AOP = mybir.AluOpType

    nc = bacc.Bacc("TRN2", target_bir_lowering=False, debug=False)

    xT_d = nc.dram_tensor("xT", [E, TPC], bf16, kind="ExternalInput").ap()
    WqT_d = nc.dram_tensor("WqT", [E, E], bf16, kind="ExternalInput").ap()
    WkT_d = nc.dram_tensor("WkT", [E, E], bf16, kind="ExternalInput").ap()
    WvT_d = nc.dram_tensor("WvT", [E, E], bf16, kind="ExternalInput").ap()
    WoT_d = nc.dram_tensor("WoT", [E, E], bf16, kind="ExternalInput").ap()
    ident_d = nc.dram_tensor("ident", [P, P], bf16, kind="ExternalInput").ap()
    mask_d = nc.dram_tensor("mask4", [P, UQ * P], bf16,
                            kind="ExternalInput").ap()
    yT_d = nc.dram_tensor("yT", [E, TPC], f32, kind="ExternalOutput").ap()

    qT_d = nc.dram_tensor("qT_scr", [E, TPC], bf16).ap()
    kT_d = nc.dram_tensor("kT_scr", [E, TPC], bf16).ap()
    vnat_d = nc.dram_tensor("vnat_scr", [TPC, E], bf16).ap()

    NE = E // P   # 16 k-tiles
    NO = E // P   # 16 o-tiles
    SC = 1.0 / math.sqrt(HD)

    with tile.TileContext(nc) as tc, ExitStack() as ctx:
        glob = ctx.enter_context(tc.tile_pool(name="glob", bufs=1))
        ident = glob.tile([P, P], bf16)
        nc.sync.dma_start(out=ident, in_=ident_d)
        mask4 = glob.tile([P, UQ, P], bf16)
        nc.sync.dma_start(out=mask4,
                          in_=mask_d.rearrange("p (j c) -> p j c", j=UQ))

        # ============ PASS A: qT/kT (feature-major) + v_nat (token-major) ====
        with nc.named_scope("passA"), \
             tc.tile_pool(name="xsb", bufs=1) as xpool, \
             tc.tile_pool(name="wA", bufs=2) as wpool, \
             tc.tile_pool(name="psA", bufs=2, space="PSUM") as pspool, \
             tc.tile_pool(name="vtps", bufs=2, space="PSUM") as vtpool, \
             tc.tile_pool(name="stA", bufs=3) as stpool, \
             tc.tile_pool(name="vstA", bufs=2) as vstpool:
            xsb = xpool.tile([P, NE, TPC], bf16)
            for e in range(NE):
                # split the x load across both DMA queues
                eng = nc.sync if e % 2 == 0 else nc.scalar
                eng.dma_start(out=xsb[:, e, :], in_=xT_d[e * P:(e + 1) * P, :])

            wmats = [WqT_d, WkT_d, WvT_d]
            outs = [qT_d, kT_d, None]
            pend_vst = None  # (vst tile, tc index, oi) awaiting transpose
            for oi in range(NO):
                wg = []
                for m in range(3):
                    wt = wpool.tile([P, NE, P], bf16, tag=f"w{m}", name="wt")
                    nc.sync.dma_start(
                        out=wt,
                        in_=wmats[m][:, oi * P:(oi + 1) * P]
                        .rearrange("(e p) o -> p e o", p=P))
                    wg.append(wt)

                def do_transpose(pend):
                    vst, ptc, poi = pend
                    vt = vtpool.tile([P, 4, P], bf16, tag="vt", name="vt")
                    for j in range(4):
                        nc.tensor.transpose(vt[:, j, :],
                                            vst[:, j * P:(j + 1) * P], ident)
                    vst2 = vstpool.tile([P, 4, P], bf16, tag="vst2",
                                        name="vst2")
                    nc.vector.tensor_copy(vst2, vt)
                    # rows are tokens ptc*TC + j*P + p ; cols poi*P..+P
                    nc.scalar.dma_start(
                        out=vnat_d[ptc * TC:(ptc + 1) * TC,
                                   poi * P:(poi + 1) * P]
                        .rearrange("(j p) d -> p j d", j=4),
                        in_=vst2)

                for tcix in range(TPC // TC):
                    for m in range(3):
                        acc = pspool.tile([P, TC], f32, tag="accA", name="acc")
                        for e in range(NE):
                            nc.tensor.matmul(
                                acc,
                                wg[m][:, e, :],
                                xsb[:, e, tcix * TC:(tcix + 1) * TC],
                                start=(e == 0), stop=(e == NE - 1))
                        if m < 2:
                            st = stpool.tile([P, TC], bf16, tag="stA",
                                             name="st")
                            nc.vector.tensor_copy(st, acc)
                            nc.scalar.dma_start(
                                out=outs[m][oi * P:(oi + 1) * P,
                                            tcix * TC:(tcix + 1) * TC],
                                in_=st)
                        else:
                            # transposes of the PREVIOUS vst go first: they
                            # must precede the new vst's buffer-slot reuse in
                            # program order, and their input has long been
                            # ready so the PE doesn't stall.
                            if pend_vst is not None:
                                do_transpose(pend_vst)
                            vst = stpool.tile([P, TC], bf16, tag="stA",
                                              name="vst")
                            nc.vector.tensor_copy(vst, acc)
                            pend_vst = (vst, tcix, oi)
            if pend_vst is not None:
                do_transpose(pend_vst)
                pend_vst = None

        # ============ PASS B: attention + Wo ============
        with nc.named_scope("passB"), \
             tc.tile_pool(name="qk", bufs=2) as qkp, \
             tc.tile_pool(name="vgp", bufs=2) as vgp, \
             tc.tile_pool(name="aop", bufs=2) as aop, \
             tc.tile_pool(name="mid", bufs=4) as mid, \
             tc.tile_pool(name="woP", bufs=2) as woP, \
             tc.tile_pool(name="yst", bufs=2) as yst, \
             tc.tile_pool(name="psS", bufs=2, space="PSUM") as psS, \
             tc.tile_pool(name="psV", bufs=2, space="PSUM") as psV, \
             tc.tile_pool(name="psT", bufs=2, space="PSUM") as psT, \
             tc.tile_pool(name="psY", bufs=2, space="PSUM") as psY:

            # --- Wo jobs: 16 oi x 16 h matmuls over the full chunk each ---
            wo_jobs = deque()

            def wo_new_job(aoT, c):
                job = {"aoT": aoT, "t0": c * TC, "pos": 0,
                       "wo": None, "wo_next": None, "yp": None}
                # issue the first weight tile load right away; it lands well
                # before the job reaches the head of the queue
                wo = woP.tile([P, NH, P], bf16, tag="wo", name="wo")
                nc.sync.dma_start(
                    out=wo,
                    in_=WoT_d[:, 0:P].rearrange("(hh p) o -> p hh o", p=P))
                job["wo_next"] = wo
                wo_jobs.append(job)

            def wo_step(nsteps):
                for _ in range(nsteps):
                    if not wo_jobs:
                        return
                    job = wo_jobs[0]
                    oi, h = divmod(job["pos"], NH)
                    job["pos"] += 1
                    if h == 0:
                        job["wo"] = job["wo_next"]
                        if oi + 1 < NO:
                            wo2 = woP.tile([P, NH, P], bf16, tag="wo",
                                           name="wo2")
                            nc.sync.dma_start(
                                out=wo2,
                                in_=WoT_d[:, (oi + 1) * P:(oi + 2) * P]
                                .rearrange("(hh p) o -> p hh o", p=P))
                            job["wo_next"] = wo2
                        else:
                            job["wo_next"] = None
                        job["yp"] = psY.tile([P, TC], f32, tag="yps",
                                             name="yps")
                    nc.tensor.matmul(
                        job["yp"], job["wo"][:, h, :],
                        job["aoT"][:, h, :],
                        start=(h == 0), stop=(h == NH - 1))
                    if h == NH - 1:
                        ys = yst.tile([P, TC], f32, tag="ys", name="ys")
                        nc.vector.tensor_copy(ys, job["yp"])
                        nc.sync.dma_start(
                            out=yT_d[oi * P:(oi + 1) * P,
                                     job["t0"]:job["t0"] + TC],
                            in_=ys)
                        if job["pos"] >= NO * NH:
                            wo_jobs.popleft()

            NSLAB = 4                # token slabs per chunk for q/k/vg loads
            SLT = TC // NSLAB        # 128 tokens per slab

            def emit_loads(c):
                """DMA chunk c's q/k slabs (fast feature-major layout) and
                vg. The matmul needs token-major contiguous (t,g) columns,
                which a strided DMA can't produce efficiently (2-byte
                gather) — so DMA feature-major slabs and relayout on
                gpsimd (emit_relayout)."""
                t0 = c * TC
                q_sl = []
                k_sl = []
                vg = vgp.tile([P, NG, HD + 2], bf16, tag="vg", name="vg")
                nc.vector.memset(vg[:, :, HD:HD + 1], 1.0)
                for s in range(NSLAB):
                    ts = t0 + s * SLT
                    qs = qkp.tile([P, NH, SLT], bf16, tag="qsl",
                                  name="q_slab", bufs=4)
                    nc.sync.dma_start(
                        out=qs,
                        in_=qT_d[:, ts:ts + SLT]
                        .rearrange("(g p) t -> p g t", p=P))
                    q_sl.append(qs)
                    ks = qkp.tile([P, NH, SLT], bf16, tag="ksl",
                                  name="k_slab", bufs=4)
                    nc.sync.dma_start(
                        out=ks,
                        in_=kT_d[:, ts:ts + SLT]
                        .rearrange("(g p) t -> p g t", p=P))
                    k_sl.append(ks)
                    # vg slab: 16 groups of 8 tokens; partition = (t8, g)
                    nc.sync.dma_start(
                        out=vg[:, s * (SLT // GS):(s + 1) * (SLT // GS),
                               0:HD],
                        in_=vnat_d[ts:ts + SLT, :]
                        .rearrange("(grp t8) (g d) -> (t8 g) grp d",
                                   t8=GS, g=NH))
                q_grp = qkp.tile([P, TC, NH], bf16, tag="qg", name="q_grp")
                k_grp = qkp.tile([P, TC, NH], bf16, tag="kg", name="k_grp")
                return {"q_sl": q_sl, "k_sl": k_sl, "vg": vg,
                        "q_grp": q_grp, "k_grp": k_grp}

            RPIECE = 2               # relayout pieces per slab
            RPT = SLT // RPIECE      # tokens per relayout piece

            def emit_relayout(st, idx):
                """Relayout piece idx into q_grp/k_grp (on gpsimd; it has no
                other pass-B work and cannot touch PSUM anyway)."""
                tensor = idx % 2
                piece = idx // 2
                s, pc = divmod(piece, RPIECE)
                tt = s * SLT + pc * RPT
                src = (st["q_sl"] if tensor == 0 else st["k_sl"])[s]
                dst = st["q_grp"] if tensor == 0 else st["k_grp"]
                nc.gpsimd.tensor_copy(
                    dst[:, tt:tt + RPT, :],
                    src[:, :, pc * RPT:(pc + 1) * RPT]
                    .rearrange("p g t -> p t g"))

            NREL = NSLAB * RPIECE * 2

            D1 = 1   # AV lag behind scores, in quads
            D2 = 2   # transpose-back lag, in quads

            loaded = emit_loads(0)
            for i in range(NREL):
                emit_relayout(loaded, i)
            nxt = None
            for c in range(TPC // TC):
                st = loaded
                q_grp, k_grp, vg = st["q_grp"], st["k_grp"], st["vg"]
                aoT = aop.tile([P, NH, TC], bf16, tag="aoT", name="aoT")

                esm_by = {}
                ao_by = {}
                for pi in range(NQ + D2):
                    if pi == 0 and c + 1 < TPC // TC:
                        nxt = emit_loads(c + 1)
                    if nxt is not None and 1 <= pi <= NREL // 2:
                        emit_relayout(nxt, 2 * (pi - 1))
                        emit_relayout(nxt, 2 * (pi - 1) + 1)
                        if pi == NREL // 2:
                            loaded = nxt
                            nxt = None
                    if pi < NQ:
                        sc = psS.tile([P, UQ, P], f32, tag="scps", name="sc")
                        # seed the whole quad's PSUM with the -1e6
                        # off-block-diagonal bias in one matmul; the score
                        # matmuls accumulate onto it and exp underflows the
                        # cross-token garbage to zero (no DVE mask op).
                        nc.tensor.matmul(
                            sc.rearrange("p j c -> p (j c)"),
                            ident, mask4.rearrange("p j c -> p (j c)"),
                            start=True, stop=False, skip_group_check=True)
                        for j in range(UQ):
                            tt = (UQ * pi + j) * GS
                            nc.tensor.matmul(
                                sc[:, j, :],
                                k_grp[:, tt:tt + GS, :]
                                .rearrange("p t h -> p (t h)"),
                                q_grp[:, tt:tt + GS, :]
                                .rearrange("p t h -> p (t h)"),
                                start=False, stop=True,
                                skip_group_check=True)
                        es = mid.tile([P, UQ, P], bf16, tag="es", name="es")
                        nc.scalar.activation(
                            out=es, in_=sc,
                            func=mybir.ActivationFunctionType.Exp,
                            scale=SC)
                        esm_by[pi] = es
                    if D1 <= pi < NQ + D1:
                        q = pi - D1
                        esm = esm_by.pop(q)
                        ao4 = mid.tile([P, UQ, HD], bf16, tag="ao4",
                                       name="ao4")
                        for half in range(2):
                            av = psV.tile([P, 2, HD + 1], f32, tag="avps",
                                          name="av")
                            for j2 in range(2):
                                j = 2 * half + j2
                                nc.tensor.matmul(
                                    av[:, j2, :], esm[:, j, :],
                                    vg[:, UQ * q + j, 0:HD + 1],
                                    start=True, stop=True)
                            iv = mid.tile([P, 2], f32, tag="iv", name="iv")
                            nc.vector.reciprocal(iv, av[:, :, HD])
                            for j2 in range(2):
                                nc.vector.tensor_scalar_mul(
                                    ao4[:, 2 * half + j2, :],
                                    av[:, j2, 0:HD], iv[:, j2:j2 + 1])
                        ao_by[q] = ao4
                    if D2 <= pi:
                        q = pi - D2
                        ao4 = ao_by.pop(q)
                        at = psT.tile([P, UQ, P], bf16, tag="atps", name="at")
                        for j in range(UQ):
                            nc.tensor.transpose(at[:, j, :], ao4[:, j, :],
                                                ident)
                        # one copy drains the whole quad (32 tokens)
                        nc.scalar.copy(
                            aoT[:, :, q * UQ * GS:(q + 1) * UQ * GS]
                            .rearrange("p h (j t) -> p j t h", j=UQ),
                            at.rearrange("p j (t h) -> p j t h", t=GS))
                    wo_step(16)   # 16 Wo matmuls per quad
                wo_new_job(aoT, c)
            # drain the last chunk's Wo
            wo_step(10 * NO * NH)

    nc.compile()
    return nc


def _get_program():
    if "nc" not in _cached:
        _cached["nc"] = _build_program()
    return _cached["nc"]


def kernel(x, Wq, Wk, Wv, Wo):
    import ml_dtypes
    from concourse.bass_utils import run_bass_kernel_spmd

    bf16 = ml_dtypes.bfloat16
    B, S, H = x.shape
    assert (B * S, H) == (NCORES * TPC, E)
    nc = _get_program()

    xf = np.ascontiguousarray(x.reshape(B * S, H))
    WqT = np.ascontiguousarray(Wq.T).astype(bf16)
    WkT = np.ascontiguousarray(Wk.T).astype(bf16)
    WvT = np.ascontiguousarray(Wv.T).astype(bf16)
    WoT = np.ascontiguousarray(Wo.T).astype(bf16)
    ident = np.eye(P, dtype=bf16)
    mask1 = (np.kron(np.eye(GS, dtype=np.float32),
                     np.ones((NH, NH), dtype=np.float32)) - 1.0) * 1e6
    mask4 = np.concatenate([mask1] * UQ, axis=1).astype(bf16)

    in_maps = []
    for i in range(NCORES):
        xT = np.ascontiguousarray(xf[i * TPC:(i + 1) * TPC, :].T).astype(bf16)
        in_maps.append({"xT": xT, "WqT": WqT, "WkT": WkT,
                        "WvT": WvT, "WoT": WoT, "ident": ident,
                        "mask4": mask4})

    import os
    trace = bool(int(os.environ.get("BASS_KERNEL_TRACE", "0")))
    res = run_bass_kernel_spmd(nc, in_maps, core_ids=list(range(NCORES)),
                               trace=trace)
    if trace:
        _cached["last_results"] = res
    parts = [res.results[i]["yT"].T for i in range(NCORES)]
    y = np.concatenate(parts, axis=0).reshape(B, S, H)
    return np.ascontiguousarray(y.astype(np.float32))


# revision 16
# speedup vs baseline: 1.1518x; 1.1518x over previous
"""Trainium2 Bass kernel for per-token multi-head self-attention (v3).

Fallback copy of the measured-1261us configuration: group-wise middle
stage (no quad batching), single full-chunk Wo jobs, all DMA on the sync
queue, aoT drain on scalar, y copies on DVE.
"""
import math
from contextlib import ExitStack

import numpy as np

NCORES = 8
E = 2048          # hidden
NH = 16           # heads
HD = 128          # head dim
TPC = 2048        # tokens per core
TC = 512          # token chunk in pass B
P = 128
GS = 8            # tokens per score group
NG = TC // GS     # groups per chunk (64)

_cached = {}


def _build_program():
    import concourse.bass as bass
    import concourse.tile as tile
    from concourse import bacc, mybir

    f32 = mybir.dt.float32
    bf16 = mybir.dt.bfloat16
    AOP = mybir.AluOpType

    nc = bacc.Bacc("TRN2", target_bir_lowering=False, debug=False)

    xT_d = nc.dram_tensor("xT", [E, TPC], bf16, kind="ExternalInput").ap()
    WqT_d = nc.dram_tensor("WqT", [E, E], bf16, kind="ExternalInput").ap()
    WkT_d = nc.dram_tensor("WkT", [E, E], bf16, kind="ExternalInput").ap()
    WvT_d = nc.dram_tensor("WvT", [E, E], bf16, kind="ExternalInput").ap()
    WoT_d = nc.dram_tensor("WoT", [E, E], bf16, kind="ExternalInput").ap()
    ident_d = nc.dram_tensor("ident", [P, P], bf16, kind="ExternalInput").ap()
    mask_d = nc.dram_tensor("mask", [P, P], bf16, kind="ExternalInput").ap()
    yT_d = nc.dram_tensor("yT", [E, TPC], f32, kind="ExternalOutput").ap()

    qT_d = nc.dram_tensor("qT_scr", [E, TPC], bf16).ap()
    kT_d = nc.dram_tensor("kT_scr", [E, TPC], bf16).ap()
    vnat_d = nc.dram_tensor("vnat_scr", [TPC, E], bf16).ap()

    NE = E // P   # 16 k-tiles
    NO = E // P   # 16 o-tiles
    SC = 1.0 / math.sqrt(HD)

    with tile.TileContext(nc) as tc, ExitStack() as ctx:
        glob = ctx.enter_context(tc.tile_pool(name="glob", bufs=1))
        ident = glob.tile([P, P], bf16)
        nc.sync.dma_start(out=ident, in_=ident_d)
        maskt = glob.tile([P, P], bf16)
        nc.sync.dma_start(out=maskt, in_=mask_d)

        # ============ PASS A: qT/kT (feature-major) + v_nat (token-major) ====
        with nc.named_scope("passA"), \
             tc.tile_pool(name="xsb", bufs=1) as xpool, \
             tc.tile_pool(name="wA", bufs=2) as wpool, \
             tc.tile_pool(name="psA", bufs=2, space="PSUM") as pspool, \
             tc.tile_pool(name="vtps", bufs=2, space="PSUM") as vtpool, \
             tc.tile_pool(name="stA", bufs=3) as stpool, \
             tc.tile_pool(name="vstA", bufs=2) as vstpool:
            xsb = xpool.tile([P, NE, TPC], bf16)
            for e in range(NE):
                nc.sync.dma_start(out=xsb[:, e, :], in_=xT_d[e * P:(e + 1) * P, :])

            wmats = [WqT_d, WkT_d, WvT_d]
            outs = [qT_d, kT_d, None]
            pend_vst = None  # (vst tile, tc index, oi) awaiting transpose
            for oi in range(NO):
                wg = []
                for m in range(3):
                    wt = wpool.tile([P, NE, P], bf16, tag=f"w{m}", name="wt")
                    nc.sync.dma_start(
                        out=wt,
                        in_=wmats[m][:, oi * P:(oi + 1) * P]
                        .rearrange("(e p) o -> p e o", p=P))
                    wg.append(wt)

                def do_transpose(pend):
                    vst, ptc, poi = pend
                    vt = vtpool.tile([P, 4, P], bf16, tag="vt", name="vt")
                    for j in range(4):
                        nc.tensor.transpose(vt[:, j, :],
                                            vst[:, j * P:(j + 1) * P], ident)
                    vst2 = vstpool.tile([P, 4, P], bf16, tag="vst2",
                                        name="vst2")
                    nc.vector.tensor_copy(vst2, vt)
                    # rows are tokens ptc*TC + j*P + p ; cols poi*P..+P
                    nc.sync.dma_start(
                        out=vnat_d[ptc * TC:(ptc + 1) * TC,
                                   poi * P:(poi + 1) * P]
                        .rearrange("(j p) d -> p j d", j=4),
                        in_=vst2)

                for tcix in range(TPC // TC):
                    for m in range(3):
                        acc = pspool.tile([P, TC], f32, tag="accA", name="acc")
                        for e in range(NE):
                            nc.tensor.matmul(
                                acc,
                                wg[m][:, e, :],
                                xsb[:, e, tcix * TC:(tcix + 1) * TC],
                                start=(e == 0), stop=(e == NE - 1))
                        if m < 2:
                            st = stpool.tile([P, TC], bf16, tag="stA",
                                             name="st")
                            nc.vector.tensor_copy(st, acc)
                            nc.sync.dma_start(
                                out=outs[m][oi * P:(oi + 1) * P,
                                            tcix * TC:(tcix + 1) * TC],
                                in_=st)
                        else:
                            # transposes of the PREVIOUS vst go first
                            if pend_vst is not None:
                                do_transpose(pend_vst)
                            vst = stpool.tile([P, TC], bf16, tag="stA",
                                              name="vst")
                            nc.vector.tensor_copy(vst, acc)
                            pend_vst = (vst, tcix, oi)
            if pend_vst is not None:
                do_transpose(pend_vst)
                pend_vst = None

        # ============ PASS B: attention + Wo ============
        with nc.named_scope("passB"), \
             tc.tile_pool(name="qk", bufs=2) as qkp, \
             tc.tile_pool(name="vgp", bufs=2) as vgp, \
             tc.tile_pool(name="aop", bufs=2) as aop, \
             tc.tile_pool(name="mid", bufs=4) as mid, \
             tc.tile_pool(name="woP", bufs=2) as woP, \
             tc.tile_pool(name="yst", bufs=2) as yst, \
             tc.tile_pool(name="psS", bufs=2, space="PSUM") as psS, \
             tc.tile_pool(name="psV", bufs=2, space="PSUM") as psV, \
             tc.tile_pool(name="psT", bufs=2, space="PSUM") as psT, \
             tc.tile_pool(name="psY", bufs=2, space="PSUM") as psY:

            wo_seq = [(oi, h) for oi in range(NO) for h in range(NH)]

            def wo_step(state, nsteps):
                for _ in range(nsteps):
                    if state is None or state["pos"] >= len(wo_seq):
                        return
                    oi, h = wo_seq[state["pos"]]
                    state["pos"] += 1
                    if h == 0:
                        if state["wo_next"] is not None:
                            state["wo"] = state["wo_next"]
                        else:
                            wo = woP.tile([P, NH, P], bf16, tag="wo",
                                          name="wo")
                            nc.sync.dma_start(
                                out=wo,
                                in_=WoT_d[:, oi * P:(oi + 1) * P]
                                .rearrange("(hh p) o -> p hh o", p=P))
                            state["wo"] = wo
                        if oi + 1 < NO:
                            wo2 = woP.tile([P, NH, P], bf16, tag="wo",
                                           name="wo2")
                            nc.sync.dma_start(
                                out=wo2,
                                in_=WoT_d[:, (oi + 1) * P:(oi + 2) * P]
                                .rearrange("(hh p) o -> p hh o", p=P))
                            state["wo_next"] = wo2
                        else:
                            state["wo_next"] = None
                        state["yp"] = psY.tile([P, TC], f32, tag="yps",
                                               name="yps")
                    nc.tensor.matmul(
                        state["yp"], state["wo"][:, h, :],
                        state["aoT"][:, h, :],
                        start=(h == 0), stop=(h == NH - 1))
                    if h == NH - 1:
                        ys = yst.tile([P, TC], f32, tag="ys", name="ys")
                        nc.vector.tensor_copy(ys, state["yp"])
                        nc.sync.dma_start(
                            out=yT_d[oi * P:(oi + 1) * P,
                                     state["t0"]:state["t0"] + TC],
                            in_=ys)

            NSLAB = 4                # token slabs per chunk for q/k/vg loads
            SLT = TC // NSLAB        # 128 tokens per slab

            def emit_loads(c):
                t0 = c * TC
                q_sl = []
                k_sl = []
                vg = vgp.tile([P, NG, HD + 2], bf16, tag="vg", name="vg")
                nc.vector.memset(vg[:, :, HD:HD + 1], 1.0)
                for s in range(NSLAB):
                    ts = t0 + s * SLT
                    qs = qkp.tile([P, NH, SLT], bf16, tag="qsl",
                                  name="q_slab", bufs=4)
                    nc.sync.dma_start(
                        out=qs,
                        in_=qT_d[:, ts:ts + SLT]
                        .rearrange("(g p) t -> p g t", p=P))
                    q_sl.append(qs)
                    ks = qkp.tile([P, NH, SLT], bf16, tag="ksl",
                                  name="k_slab", bufs=4)
                    nc.sync.dma_start(
                        out=ks,
                        in_=kT_d[:, ts:ts + SLT]
                        .rearrange("(g p) t -> p g t", p=P))
                    k_sl.append(ks)
                    nc.sync.dma_start(
                        out=vg[:, s * (SLT // GS):(s + 1) * (SLT // GS),
                               0:HD],
                        in_=vnat_d[ts:ts + SLT, :]
                        .rearrange("(grp t8) (g d) -> (t8 g) grp d",
                                   t8=GS, g=NH))
                q_grp = qkp.tile([P, TC, NH], bf16, tag="qg", name="q_grp")
                k_grp = qkp.tile([P, TC, NH], bf16, tag="kg", name="k_grp")
                return {"q_sl": q_sl, "k_sl": k_sl, "vg": vg,
                        "q_grp": q_grp, "k_grp": k_grp}

            RPIECE = 2               # relayout pieces per slab
            RPT = SLT // RPIECE      # tokens per relayout piece

            def emit_relayout(st, idx):
                tensor = idx % 2
                piece = idx // 2
                s, pc = divmod(piece, RPIECE)
                tt = s * SLT + pc * RPT
                src = (st["q_sl"] if tensor == 0 else st["k_sl"])[s]
                dst = st["q_grp"] if tensor == 0 else st["k_grp"]
                eng = nc.vector if tensor == 0 else nc.gpsimd
                eng.tensor_copy(
                    dst[:, tt:tt + RPT, :],
                    src[:, :, pc * RPT:(pc + 1) * RPT]
                    .rearrange("p g t -> p t g"))

            NREL = NSLAB * RPIECE * 2

            D1 = 2   # AV lag behind scores, in groups
            D2 = 3   # transpose-back lag, in groups

            prev = None
            loaded = emit_loads(0)
            for i in range(NREL):
                emit_relayout(loaded, i)
            nxt = None
            for c in range(TPC // TC):
                t0 = c * TC
                st = loaded
                q_grp, k_grp, vg = st["q_grp"], st["k_grp"], st["vg"]
                aoT = aop.tile([P, NH, TC], bf16, tag="aoT", name="aoT")

                esm_by = {}
                ao_by = {}
                for g in range(NG + D2):
                    if g == 4 and c + 1 < TPC // TC:
                        nxt = emit_loads(c + 1)
                    if nxt is not None and 8 <= g < 8 + 2 * NREL \
                            and (g - 8) % 2 == 0:
                        emit_relayout(nxt, (g - 8) // 2)
                        if g == 8 + 2 * NREL - 2:
                            loaded = nxt
                            nxt = None
                    if g < NG:
                        tt = g * GS
                        sc = psS.tile([P, P], f32, tag="scps", name="sc")
                        nc.tensor.matmul(
                            sc,
                            k_grp[:, tt:tt + GS, :]
                            .rearrange("p t h -> p (t h)"),
                            q_grp[:, tt:tt + GS, :]
                            .rearrange("p t h -> p (t h)"),
                            start=True, stop=True)
                        es = mid.tile([P, P], bf16, tag="es", name="es")
                        nc.scalar.activation(
                            out=es, in_=sc,
                            func=mybir.ActivationFunctionType.Exp,
                            scale=SC)
                        esm = mid.tile([P, P], bf16, tag="esm", name="esm")
                        nc.vector.scalar_tensor_tensor(
                            esm, es, 1.0, maskt,
                            op0=AOP.bypass, op1=AOP.mult)
                        esm_by[g] = esm
                    if D1 <= g < NG + D1:
                        i = g - D1
                        av = psV.tile([P, HD + 1], f32, tag="avps", name="av")
                        nc.tensor.matmul(av, esm_by.pop(i),
                                         vg[:, i, 0:HD + 1],
                                         start=True, stop=True)
                        iv = mid.tile([P, 1], f32, tag="iv", name="iv")
                        nc.vector.reciprocal(iv, av[:, HD:HD + 1])
                        ao = mid.tile([P, HD], bf16, tag="ao", name="ao")
                        nc.vector.tensor_scalar_mul(ao, av[:, 0:HD], iv)
                        ao_by[i] = ao
                    if D2 <= g:
                        i = g - D2
                        at = psT.tile([P, P], bf16, tag="atps", name="at")
                        nc.tensor.transpose(at, ao_by.pop(i), ident)
                        nc.scalar.copy(
                            aoT[:, :, i * GS:(i + 1) * GS]
                            .rearrange("p h t -> p t h"),
                            at.rearrange("p (t h) -> p t h", t=GS))
                    wo_step(prev, 4)

                wo_step(prev, len(wo_seq))
                prev = {"pos": 0, "aoT": aoT, "t0": t0,
                        "wo": None, "wo_next": None, "yp": None}
            wo_step(prev, len(wo_seq))

    nc.compile()
    return nc


def _get_program():
    if "nc" not in _cached:
        _cached["nc"] = _build_program()
    return _cached["nc"]


def kernel(x, Wq, Wk, Wv, Wo):
    import ml_dtypes
    from concourse.bass_utils import run_bass_kernel_spmd

    bf16 = ml_dtypes.bfloat16
    B, S, H = x.shape
    assert (B * S, H) == (NCORES * TPC, E)
    nc = _get_program()

    xf = np.ascontiguousarray(x.reshape(B * S, H))
    WqT = np.ascontiguousarray(Wq.T).astype(bf16)
    WkT = np.ascontiguousarray(Wk.T).astype(bf16)
    WvT = np.ascontiguousarray(Wv.T).astype(bf16)
    WoT = np.ascontiguousarray(Wo.T).astype(bf16)
    ident = np.eye(P, dtype=bf16)
    mask = np.kron(np.eye(GS, dtype=np.float32),
                   np.ones((NH, NH), dtype=np.float32)).astype(bf16)

    in_maps = []
    for i in range(NCORES):
        xT = np.ascontiguousarray(xf[i * TPC:(i + 1) * TPC, :].T).astype(bf16)
        in_maps.append({"xT": xT, "WqT": WqT, "WkT": WkT,
                        "WvT": WvT, "WoT": WoT, "ident": ident,
                        "mask": mask})

    import os
    trace = bool(int(os.environ.get("BASS_KERNEL_TRACE", "0")))
    res = run_bass_kernel_spmd(nc, in_maps, core_ids=list(range(NCORES)),
                               trace=trace)
    if trace:
        _cached["last_results"] = res
    parts = [res.results[i]["yT"].T for i in range(NCORES)]
    y = np.concatenate(parts, axis=0).reshape(B, S, H)
    return np.ascontiguousarray(y.astype(np.float32))
